# revision 1
# baseline (speedup 1.0000x reference)
"""GATv2 3-layer GNN kernel for TRN2 (Bass/Tile), 8-core SPMD.

Strategy (see spec sharding_hint): graph-partition over destination nodes.
- Host: nodes assigned to NC*NB blocks of <=128 slots, edge lists per block
  padded to a uniform chunk count; all indices translated to "slot space".
- Device (per core, SPMD): per layer
    dense:  gl = h @ Wl (+bias) for own nodes -> AllGather -> gl table in HBM
            gr = h @ Wr (+bias) for own nodes (stays in SBUF)
    edges:  per block: dma_gather gl[src] rows; per 128-edge chunk build
            one-hot A [e,dst] / AT [dst,e] from dstrel; PE: tmp = AT.T@gr_blk
            + I.T@gl_src (PSUM); ACT lrelu; DVE att-mul; reduce -> score;
            ACT exp; DVE expand ex; values = ex*gl_src; PE scatter:
            out_blk += A.T @ [values | ex]  (num and denom in one matmul)
    node:   h = elu(num/den + bias)  (layer3: sigmoid -> output)
Normalization by the softmax denominator happens per node after aggregation
(identical math to per-edge alpha; segment-max is skipped -- scores are O(6)).
"""

import numpy as np
import ml_dtypes

import concourse.bass as bass
import concourse.mybir as mybir
from concourse import tile

BF16 = mybir.dt.bfloat16
F32 = mybir.dt.float32
I16 = mybir.dt.int16

AF = mybir.ActivationFunctionType
ALU = mybir.AluOpType
AX = mybir.AxisListType

NEG_SLOPE = 0.2


# ---------------------------------------------------------------- host prep

def assign_blocks(dst, n_nodes, n_bins):
    """Greedy balanced assignment of nodes to bins (<=128 nodes each),
    balancing total edge count per bin. Returns slot_of_node [N] (global slot
    id = bin*128 + pos) and node_of_slot [n_bins*128] (-1 = empty)."""
    deg = np.bincount(dst, minlength=n_nodes)
    order = np.argsort(-deg, kind="stable")
    load = np.zeros(n_bins, dtype=np.int64)
    count = np.zeros(n_bins, dtype=np.int64)
    slot_of_node = np.full(n_nodes, -1, dtype=np.int64)
    node_of_slot = np.full(n_bins * 128, -1, dtype=np.int64)
    for n in order:
        cand = np.where(count < 128)[0]
        b = cand[np.argmin(load[cand])]
        slot = b * 128 + count[b]
        slot_of_node[n] = slot
        node_of_slot[slot] = n
        count[b] += 1
        load[b] += deg[n]
    return slot_of_node, node_of_slot


def prep_host(x, edge_index, n_cores, nb, chunk_group=4):
    """Build per-core tables. Returns dict of host data."""
    n_nodes = x.shape[0]
    n_bins = n_cores * nb
    src, dst = np.asarray(edge_index[0]), np.asarray(edge_index[1])
    slot_of_node, node_of_slot = assign_blocks(dst, n_nodes, n_bins)

    sslot = slot_of_node[src]          # source slot per edge
    dslot = slot_of_node[dst]
    dbin = dslot // 128
    drel = dslot % 128

    # group edges by destination bin
    ord_ = np.argsort(dbin, kind="stable")
    sslot, drel, dbin = sslot[ord_], drel[ord_], dbin[ord_]
    counts = np.bincount(dbin, minlength=n_bins)
    k_chunks = int(np.ceil(counts.max() / 128))
    k_chunks = int(np.ceil(k_chunks / chunk_group) * chunk_group)
    eb = k_chunks * 128                 # padded edges per bin
    e_core = nb * eb                    # edges per core

    src_pad = np.zeros((n_bins, eb), dtype=np.int64)
    drel_pad = np.full((n_bins, eb), -1.0, dtype=np.float32)
    ofs = np.concatenate([[0], np.cumsum(counts)])
    for b in range(n_bins):
        c = counts[b]
        src_pad[b, :c] = sslot[ofs[b]:ofs[b] + c]
        drel_pad[b, :c] = drel[ofs[b]:ofs[b] + c]

    per_core = []
    for c in range(n_cores):
        s = src_pad[c * nb:(c + 1) * nb].reshape(-1)          # [e_core]
        d = drel_pad[c * nb:(c + 1) * nb].reshape(-1)         # [e_core]
        # gather idx: wrapped in 16 partitions, replicated x8
        idx16 = s.astype(np.int16).reshape(-1, 16).T          # [16, e/16]
        idx16 = np.tile(idx16, (8, 1)).copy()                 # [128, e/16]
        # dstrel per chunk, partition-major: [128, n_chunks]
        dcol = d.reshape(-1, 128).T.astype(np.float32).copy() # [128, nchunks]
        # dstrel rows replicated across partitions for the AT build:
        # [128, nchunks, 128] int16 (DRAM-resident; streamed per block)
        dr = d.reshape(-1, 128).astype(np.int16)              # [nchunks,128]
        drow = np.ascontiguousarray(
            np.broadcast_to(dr[None, :, :], (128,) + dr.shape))
        per_core.append(dict(idx16=idx16, dcol=dcol, drow=drow))

    return dict(
        slot_of_node=slot_of_node, node_of_slot=node_of_slot,
        k_chunks=k_chunks, e_core=e_core, per_core=per_core,
        n_bins=n_bins,
    )


def pack_weights(inp, meta, n_cores, nb):
    """Pack weights/constants shared by all cores (host-side, numpy).
    Returns dict name -> np.ndarray to be fed as kernel inputs."""
    bf = ml_dtypes.bfloat16
    node_of_slot = meta["node_of_slot"]
    slots = node_of_slot.shape[0]
    x = np.asarray(inp["x"])
    n, in_ch = x.shape

    xs = np.zeros((slots, in_ch), dtype=np.float32)
    valid = node_of_slot >= 0
    xs[valid] = x[node_of_slot[valid]]

    out = {}
    # per-core transposed x slice  [128, nb*128]
    per_core_x = []
    sl_per_core = slots // n_cores
    for c in range(n_cores):
        per_core_x.append(
            np.ascontiguousarray(xs[c * sl_per_core:(c + 1) * sl_per_core].T)
            .astype(bf))
    out["__percore__xT"] = per_core_x

    def b(a):
        return np.asarray(a, dtype=bf)

    for li, (wl, bl, wr, br, att, bias, heads, ch) in enumerate([
        (inp["Wl1"], inp["bl1"], inp["Wr1"], inp["br1"], inp["att1"], inp["bias1"], 8, 32),
        (inp["Wl2"], inp["bl2"], inp["Wr2"], inp["br2"], inp["att2"], inp["bias2"], 8, 32),
        (inp["Wl3"], inp["bl3"], inp["Wr3"], inp["br3"], inp["att3"], inp["bias3"], 1, 64),
    ], start=1):
        wl = np.asarray(wl, np.float32); wr = np.asarray(wr, np.float32)
        d2 = heads * ch
        if li == 3:
            # pad out channels 64 -> 128 for the gl3 gather table
            wl = np.concatenate([wl, np.zeros((wl.shape[0], 128 - d2), np.float32)], 1)

        def kblk(w):
            # [in, out] -> [128, kb, out]
            inch = w.shape[0]
            kb = (inch + 127) // 128
            wp = np.zeros((kb * 128, w.shape[1]), np.float32)
            wp[:inch] = w
            return np.ascontiguousarray(wp.reshape(kb, 128, -1).transpose(1, 0, 2))

        out[f"Wl{li}"] = b(kblk(wl))
        out[f"Wr{li}"] = b(kblk(wr))
        def brd(v, pad_to=None):
            v = np.asarray(v, np.float32).reshape(1, -1)
            if pad_to is not None and v.shape[1] < pad_to:
                v = np.concatenate(
                    [v, np.zeros((1, pad_to - v.shape[1]), np.float32)], 1)
            return np.ascontiguousarray(np.tile(v, (128, 1)))

        out[f"bl{li}"] = brd(bl, 128 if li == 3 else None)
        out[f"br{li}"] = brd(br)
        out[f"obias{li}"] = brd(bias)
        attrow = np.asarray(att, np.float32).reshape(1, d2)
        out[f"att{li}"] = b(np.tile(attrow, (128, 1)))
    out["ident"] = b(np.eye(128, dtype=np.float32))
    out["iota_row"] = np.tile(
        np.arange(128, dtype=np.int16)[None, :], (128, 1)).copy()
    out["iota_col"] = np.arange(128, dtype=np.float32)[:, None].copy()
    return out


CONST_ORDER = [
    "ident", "iota_row", "iota_col", "xT", "idx16", "dcol",
    "Wl1", "Wr1", "bl1", "br1", "obias1", "att1",
    "Wl2", "Wr2", "bl2", "br2", "obias2", "att2",
    "Wl3", "Wr3", "bl3", "br3", "obias3", "att3",
]


def make_core_inputs(packed, meta, core):
    """Assemble per-core kernel inputs: one const blob + the drow table.
    Returns (inputs_dict, blob_offsets)."""
    pc = meta["per_core"][core]
    consts = {}
    for name in CONST_ORDER:
        if name == "xT":
            consts[name] = packed["__percore__xT"][core]
        elif name == "idx16":
            consts[name] = pc["idx16"]
        elif name == "dcol":
            consts[name] = pc["dcol"]
        else:
            consts[name] = packed[name]
    blob, offsets = build_blob(consts)
    return {"blob": blob, "drow": np.ascontiguousarray(pc["drow"])}, offsets


def build_blob(consts):
    """Pack {name: [128, ...] array} into one [128, B] uint8 blob + offset map.
    Each array must have partition dim 128; free dims flattened."""
    offsets = {}
    parts = []
    off = 0
    for name, arr in consts.items():
        assert arr.shape[0] == 128, (name, arr.shape)
        flat = np.ascontiguousarray(arr).reshape(128, -1)
        by = flat.view(np.uint8).reshape(128, -1)
        pad = (-by.shape[1]) % 4
        if pad:
            by = np.concatenate(
                [by, np.zeros((128, pad), np.uint8)], axis=1)
        offsets[name] = (off, arr.dtype, arr.shape[1:])
        parts.append(by)
        off += by.shape[1]
    return np.concatenate(parts, axis=1), offsets


# ---------------------------------------------------------------- kernel

class Cfg:
    def __init__(self, n_cores, nb, k_chunks, grp=4):
        self.n_cores = n_cores
        self.nb = nb                  # blocks per core
        self.k_chunks = k_chunks      # 128-edge chunks per block
        self.grp = grp                # chunks per group
        self.slots = n_cores * nb * 128
        self.own = nb * 128
        self.e_core = nb * k_chunks * 128


def build_kernel(tc, outs, ins, cfg: Cfg):
    nc = tc.nc
    NB, K, G = cfg.nb, cfg.k_chunks, cfg.grp
    NGRP = K // G
    OWN = cfg.own
    SLOTS = cfg.slots

    out_dram = outs["out"]
    IN = 128

    # layer configs: (d2, heads, ch, table_cols, in_ch)
    layers = [
        dict(li=1, heads=8, ch=32, d2=256, tab=256, inch=IN, kb=1),
        dict(li=2, heads=8, ch=32, d2=256, tab=256, inch=256, kb=2),
        dict(li=3, heads=1, ch=64, d2=64, tab=128, inch=256, kb=2),
    ]

    from contextlib import ExitStack
    ctx = ExitStack()
    import os as _os
    _nl = int(_os.environ.get("GAT_LAYERS", "3"))
    _ph = _os.environ.get("GAT_PHASE", "all")
    cc = ctx.enter_context(tc.tile_pool(name="const", bufs=1))
    dram = ctx.enter_context(tc.tile_pool(name="dram", bufs=1, space="DRAM"))
    work = ctx.enter_context(tc.tile_pool(name="work", bufs=2))
    psum = ctx.enter_context(tc.tile_pool(name="psum", bufs=2, space="PSUM"))
    psum_out = ctx.enter_context(
        tc.tile_pool(name="psum_out", bufs=2, space="PSUM"))
    psum_d = ctx.enter_context(tc.tile_pool(name="psum_d", bufs=1, space="PSUM"))
    gath_pool = ctx.enter_context(tc.tile_pool(name="gath", bufs=2))

    # ---------- load all constants with ONE DMA from the packed blob
    np2dt = {
        np.dtype(np.float32): F32,
        np.dtype(np.int16): I16,
        np.dtype("bfloat16"): BF16,
    }
    blob_ap = ins["blob"]
    blob = cc.tile([128, blob_ap.shape[1]], mybir.dt.uint8, tag="blob")
    nc.sync.dma_start(blob[:], blob_ap)

    def cview(name):
        off, dt, shape = cfg.blob_offsets[name]
        dtm = np2dt[np.dtype(dt)]
        n = int(np.prod(shape)) if shape else 1
        v = blob[:, off:off + n * np.dtype(dt).itemsize].bitcast(dtm)
        if len(shape) == 2:
            v = v.rearrange("p (a b) -> p a b", b=shape[1])
        return v

    ident = cview("ident")
    iota_row = cview("iota_row")           # [128,128] i16: value = col
    iota_col = cview("iota_col")           # [128,1] f32: value = partition
    idx16 = cview("idx16")
    dcol = cview("dcol")
    drow_dram = ins["drow"]                # [128, nch, 128] i16 (DRAM)
    xT = cview("xT")
    wt = {}
    for l in layers:
        li = l["li"]
        for nm in (f"Wl{li}", f"Wr{li}", f"bl{li}", f"br{li}",
                   f"obias{li}", f"att{li}"):
            wt[nm] = cview(nm)

    # persistent h state (own nodes)
    h_sb = cc.tile([128, NB, 256], BF16, tag="h_sb")
    hT = cc.tile([128, 2, OWN], BF16, tag="hT")
    gr_sb = cc.tile([128, NB, 256], BF16, tag="gr_sb")

    # DRAM: gl shard + allgather output per layer
    gl_shard = {
        l["li"]: dram.tile([OWN, l["tab"]], BF16, name=f"gl_shard{l['li']}")
        for l in layers
    }
    gl_full = {
        l["li"]: dram.tile([SLOTS, l["tab"]], BF16, addr_space="Shared",
                           name=f"gl_full{l['li']}")
        for l in layers
    }

    replica_groups = [list(range(cfg.n_cores))]

    for l in layers[:_nl]:
        li, heads, ch, d2, tab, inch, kb = (
            l["li"], l["heads"], l["ch"], l["d2"], l["tab"], l["inch"], l["kb"])

        # ---------------- dense phase ----------------
        if li > 1:
            # hT <- transpose(h_sb)
            for b in range(NB):
                for k in range(2):
                    pt = psum_d.tile([128, 128], BF16, tag="pt")
                    nc.tensor.transpose(
                        pt[:], h_sb[:, b, k * 128:(k + 1) * 128], ident[:])
                    nc.vector.tensor_copy(
                        hT[:, k, b * 128:(b + 1) * 128], pt[:])

        def lhsT_blk(kbi, b):
            if li == 1:
                return xT[:, b * 128:(b + 1) * 128]
            return hT[:, kbi, b * 128:(b + 1) * 128]

        gl_blk = {}
        for b in range(NB):
            for (wn, bn, store_gr) in ((f"Wl{li}", f"bl{li}", False),
                                       (f"Wr{li}", f"br{li}", True)):
                cols = d2 if store_gr else tab
                pg = psum_d.tile([128, 256], F32, tag="pg")
                for kbi in range(kb):
                    nc.tensor.matmul(
                        pg[:, 0:cols], lhsT_blk(kbi, b),
                        wt[wn][:, kbi, 0:cols],
                        start=(kbi == 0), stop=(kbi == kb - 1))
                if store_gr:
                    nc.vector.tensor_tensor(
                        gr_sb[:, b, 0:cols], pg[:, 0:cols],
                        wt[bn][:, 0:cols], ALU.add)
                else:
                    t = work.tile([128, tab], BF16, tag="gl_blk")
                    nc.vector.tensor_tensor(
                        t[:, 0:cols], pg[:, 0:cols],
                        wt[bn][:, 0:cols], ALU.add)
                    nc.sync.dma_start(
                        gl_shard[li][b * 128:(b + 1) * 128, :], t[:])
        # allgather gl table
        nc.gpsimd.collective_compute(
            "AllGather", ALU.bypass,
            ins=[gl_shard[li].opt()], outs=[gl_full[li].opt()],
            replica_groups=replica_groups)
        if f"gl{li}" in outs:   # debug: dump gathered table
            dbg = work.tile([128, SLOTS // 128, tab], BF16, tag=f"dbg{li}")
            nc.sync.dma_start(
                dbg[:], gl_full[li].rearrange("(n p) c -> p n c", p=128))
            dbgf = work.tile([128, SLOTS // 128, tab], F32, tag=f"dbgf{li}")
            nc.vector.tensor_copy(dbgf[:], dbg[:])
            nc.sync.dma_start(
                outs[f"gl{li}"].rearrange("(n p) c -> p n c", p=128), dbgf[:])

        # ---------------- edge phase ----------------
        att = wt[f"att{li}"]
        obias = wt[f"obias{li}"]
        ech = K * 128 // 16            # idx16 cols per block
        for b in (range(NB) if _ph == "all" else []):
            gt = gath_pool.tile([128, K, tab], BF16, tag="gath")
            # split into <=1024-index sub-gathers: larger single calls
            # (4096 idxs) abort/hang the SWDGE path on this runtime
            GSUB = 8                     # chunks (of 128 edges) per gather
            for gs in range(0, K, GSUB):
                kk = min(GSUB, K - gs)
                nc.gpsimd.dma_gather(
                    gt[:, gs:gs + kk, :], gl_full[li],
                    idx16[:, b * ech + gs * 8:b * ech + (gs + kk) * 8],
                    num_idxs=kk * 128, num_idxs_reg=kk * 128,
                    elem_size=tab, queue_num=0)
            drb = gath_pool.tile([128, K, 128], I16, tag="drb")
            nc.sync.dma_start(drb[:], drow_dram[:, b * K:(b + 1) * K, :])
            po = psum_out.tile([128, 512], F32, tag="po")
            for g in range(NGRP):
                A4 = work.tile([128, G, 128], BF16, tag="A4")
                AT4 = work.tile([128, G, 128], BF16, tag="AT4")
                for j in range(G):
                    ci = b * K + g * G + j      # global chunk index (core)
                    nc.vector.tensor_scalar(
                        A4[:, j, :], iota_row[:],
                        dcol[:, ci:ci + 1], None, op0=ALU.is_equal)
                    nc.vector.tensor_scalar(
                        AT4[:, j, :], drb[:, g * G + j, :],
                        iota_col[:, 0:1], None, op0=ALU.is_equal)
                tp = psum.tile([128, G, 256], F32, tag="tp")
                for j in range(G):
                    nc.tensor.matmul(
                        tp[:, j, 0:d2], AT4[:, j, :], gr_sb[:, b, 0:d2],
                        start=True, stop=False)
                    nc.tensor.matmul(
                        tp[:, j, 0:d2], ident[:],
                        gt[:, g * G + j, 0:d2], start=False, stop=True)
                tmpc = work.tile([128, G, 256], BF16, tag="tmpc")
                nc.scalar.activation(tmpc[:, :, 0:d2], tp[:, :, 0:d2], AF.Copy)
                tmp = work.tile([128, G, 256], BF16, tag="tmp")
                nc.vector.scalar_tensor_tensor(
                    tmp[:, :, 0:d2], tmpc[:, :, 0:d2], NEG_SLOPE,
                    tmpc[:, :, 0:d2], op0=ALU.mult, op1=ALU.max)
                t2 = work.tile([128, G, 256], BF16, tag="t2")
                nc.vector.tensor_tensor(
                    t2[:, :, 0:d2], tmp[:, :, 0:d2],
                    att[:, 0:d2].unsqueeze(1).broadcast_to((128, G, d2)),
                    ALU.mult)
                score = work.tile([128, G, 8], F32, tag="score")
                nc.vector.tensor_reduce(
                    score[:, :, 0:heads],
                    t2[:, :, 0:d2].rearrange("p g (h c) -> p g h c", c=ch),
                    axis=AX.X, op=ALU.add)
                ex = work.tile([128, G, 8], BF16, tag="ex")
                nc.scalar.activation(
                    ex[:, :, 0:heads], score[:, :, 0:heads], AF.Exp)
                # expand ex per head over its channels (strided doubling)
                rhs = work.tile([128, G, 272], BF16, tag="rhs")
                exe = work.tile([128, G, 256], BF16, tag="exe")
                exe4 = exe[:, :, 0:d2].rearrange(
                    "p g (h c) -> p g h c", c=ch)
                nc.vector.tensor_copy(
                    exe4[:, :, :, 0:1], ex[:, :, 0:heads].unsqueeze(3))
                w = 1
                while w < ch:
                    nc.vector.tensor_copy(
                        exe4[:, :, :, w:2 * w], exe4[:, :, :, 0:w])
                    w *= 2
                nc.vector.tensor_tensor(
                    rhs[:, :, 0:d2], gt[:, g * G:(g + 1) * G, 0:d2],
                    exe[:, :, 0:d2], ALU.mult)
                nc.vector.tensor_copy(
                    rhs[:, :, d2:d2 + heads], ex[:, :, 0:heads])
                for j in range(G):
                    nc.tensor.matmul(
                        po[:, 0:d2 + heads], A4[:, j, :],
                        rhs[:, j, 0:d2 + heads],
                        start=(g == 0 and j == 0),
                        stop=(g == NGRP - 1 and j == G - 1))
                if b == 0 and g == 0 and f"dbg_l{li}" in outs:
                    dl = outs[f"dbg_l{li}"]
                    ofs = 0
                    for nm, tl, w in (("gt", gt, tab), ("A", A4, 128),
                                      ("AT", AT4, 128), ("t2", t2, 256),
                                      ("sc", score, 8), ("rhs", rhs, 264)):
                        db = work.tile([128, G, w], F32, tag=f"db_{nm}",
                                       name=f"db_{nm}")
                        src_ap = tl[:, 0:G, 0:w]
                        nc.vector.tensor_copy(db[:], src_ap)
                        nc.sync.dma_start(
                            dl[:, ofs:ofs + G * w],
                            db[:].rearrange("p g w -> p (g w)"))
                        ofs += G * w
            # -------- block epilogue: normalize + bias (+elu / sigmoid)
            den = work.tile([128, 8], F32, tag="den")
            nc.vector.tensor_scalar(
                den[:, 0:heads], po[:, d2:d2 + heads], 1e-16, None,
                op0=ALU.add)
            rec = work.tile([128, 8], F32, tag="rec")
            nc.vector.reciprocal(rec[:, 0:heads], den[:, 0:heads])
            recx = work.tile([128, 256], F32, tag="recx")
            recx4 = recx[:, 0:d2].rearrange("p (h c) -> p h c", c=ch)
            nc.vector.tensor_copy(
                recx4[:, :, 0:1], rec[:, 0:heads].unsqueeze(2))
            w = 1
            while w < ch:
                nc.vector.tensor_copy(recx4[:, :, w:2 * w], recx4[:, :, 0:w])
                w *= 2
            hx = work.tile([128, 256], F32, tag="hx")
            nc.vector.tensor_tensor(
                hx[:, 0:d2], po[:, 0:d2], recx[:, 0:d2], ALU.mult)
            nc.vector.tensor_tensor(
                hx[:, 0:d2], hx[:, 0:d2], obias[:, 0:d2], ALU.add)
            if li < 3:
                m0 = work.tile([128, 256], F32, tag="m0")
                nc.vector.tensor_scalar(
                    m0[:, 0:d2], hx[:, 0:d2], 0.0, None, op0=ALU.min)
                e0 = work.tile([128, 256], F32, tag="e0")
                nc.scalar.activation(e0[:, 0:d2], m0[:, 0:d2], AF.Exp)
                nc.vector.scalar_tensor_tensor(
                    h_sb[:, b, 0:d2], e0[:, 0:d2], -1.0, hx[:, 0:d2],
                    op0=ALU.add, op1=ALU.max)
                if f"h{li}" in outs:
                    hdbg = work.tile([128, 256], F32, tag="hdbg")
                    nc.vector.tensor_copy(hdbg[:, 0:d2], h_sb[:, b, 0:d2])
                    nc.sync.dma_start(
                        outs[f"h{li}"][b * 128:(b + 1) * 128, :],
                        hdbg[:, 0:d2])
            else:
                so = work.tile([128, 64], F32, tag="so")
                nc.scalar.activation(so[:, 0:d2], hx[:, 0:d2], AF.Sigmoid)
                nc.sync.dma_start(
                    out_dram[b * 128:(b + 1) * 128, :], so[:, 0:d2])
    ctx.close()


# ================================================================ entry point

N_NODES, N_EDGES = 10000, 320000
OUT_CH = 64
N_CORES_K = 8
NB_K = 10

_KERNEL_CACHE = {}


def _build_program(cfg, shapes_dtypes):
    import concourse.bacc as bacc
    nc = bacc.Bacc("TRN2", target_bir_lowering=False, debug=False,
                   enable_asserts=False, num_devices=cfg.n_cores)
    ins = {}
    for name, (shape, dt) in shapes_dtypes.items():
        ins[name] = nc.dram_tensor(name, list(shape), dt, kind="ExternalInput").ap()
    out = nc.dram_tensor("out", [cfg.own, OUT_CH], F32, kind="ExternalOutput").ap()
    with tile.TileContext(nc) as tc:
        build_kernel(tc, {"out": out}, ins, cfg)
    nc.compile()
    return nc


_PREP_CACHE = {}


def kernel(**inputs):
    """Full-input entry: shard across 8 NeuronCores, run, gather."""
    from concourse.bass_utils import run_bass_kernel_spmd

    x = np.asarray(inputs["x"], np.float32)
    edge_index = np.asarray(inputs["edge_index"])
    ekey = hash(edge_index.tobytes())
    if ekey in _PREP_CACHE:
        meta = _PREP_CACHE[ekey]
    else:
        meta = prep_host(x, edge_index, N_CORES_K, NB_K)
        _PREP_CACHE.clear()
        _PREP_CACHE[ekey] = meta
    cfg = Cfg(N_CORES_K, NB_K, meta["k_chunks"])
    packed = pack_weights(inputs, meta, N_CORES_K, NB_K)

    in_maps = []
    for c in range(N_CORES_K):
        d, offsets = make_core_inputs(packed, meta, c)
        cfg.blob_offsets = offsets
        in_maps.append(d)

    key = (cfg.k_chunks, in_maps[0]["blob"].shape[1])
    if key not in _KERNEL_CACHE:
        shapes_dtypes = {
            name: (arr.shape, mybir.dt.from_np(arr.dtype))
            for name, arr in in_maps[0].items()
        }
        _KERNEL_CACHE[key] = _build_program(cfg, shapes_dtypes)
    nc = _KERNEL_CACHE[key]

    res = run_bass_kernel_spmd(
        nc, in_maps, core_ids=list(range(N_CORES_K)), trace=False)
    kernel.last_results = res

    nos = meta["node_of_slot"]
    full = np.zeros((N_NODES, OUT_CH), np.float32)
    for c in range(N_CORES_K):
        sl = nos[c * cfg.own:(c + 1) * cfg.own]
        v = sl >= 0
        full[sl[v]] = res.results[c]["out"][v]
    return full


kernel.last_results = None



# revision 4
# speedup vs baseline: 16.5196x; 16.5196x over previous
"""GATv2 3-layer GNN kernel for TRN2 (Bass/Tile), 8-core SPMD.

Strategy (see spec sharding_hint): graph-partition over destination nodes.
- Host: nodes assigned to NC*NB blocks of <=128 slots, edge lists per block
  padded to a uniform chunk count; all indices translated to "slot space".
- Device (per core, SPMD): per layer
    dense:  gl = h @ Wl (+bias) for own nodes -> AllGather -> gl table in HBM
            gr = h @ Wr (+bias) for own nodes (stays in SBUF)
    edges:  per block: dma_gather gl[src] rows; per 128-edge chunk build
            one-hot A [e,dst] from dstrel (iota compare); AT = PE-transpose(A);
            PE: tmp = AT.T@gr_blk + I.T@gl_src (PSUM); ACT lrelu; DVE att-mul;
            reduce -> score; ACT exp; rhs = ex*gl_src (broadcast AP);
            PE scatter: out_blk += A.T @ [values | ex]  (num+denom in 1 matmul)
    node:   h = elu(num/den + bias)  (layer3: sigmoid -> output, fp16)
Normalization by the softmax denominator happens per node after aggregation
(identical math to per-edge alpha; segment-max is skipped -- scores are O(6)).

Runtime: the jitted shard_map callable and all device-resident inputs are
cached keyed by input content hash; a warm call is a single PJRT dispatch.
"""

import hashlib

import numpy as np
import ml_dtypes

import concourse.bass as bass
import concourse.mybir as mybir
from concourse import tile

BF16 = mybir.dt.bfloat16
F32 = mybir.dt.float32
F16 = mybir.dt.float16
I16 = mybir.dt.int16

AF = mybir.ActivationFunctionType
ALU = mybir.AluOpType
AX = mybir.AxisListType

NEG_SLOPE = 0.2


# ---------------------------------------------------------------- host prep

def assign_blocks(dst, n_nodes, n_bins):
    """Greedy balanced assignment of nodes to bins (<=128 nodes each),
    balancing total edge count per bin. Returns slot_of_node [N] (global slot
    id = bin*128 + pos) and node_of_slot [n_bins*128] (-1 = empty)."""
    deg = np.bincount(dst, minlength=n_nodes)
    order = np.argsort(-deg, kind="stable")
    load = np.zeros(n_bins, dtype=np.int64)
    count = np.zeros(n_bins, dtype=np.int64)
    slot_of_node = np.full(n_nodes, -1, dtype=np.int64)
    node_of_slot = np.full(n_bins * 128, -1, dtype=np.int64)
    for n in order:
        cand = np.where(count < 128)[0]
        b = cand[np.argmin(load[cand])]
        slot = b * 128 + count[b]
        slot_of_node[n] = slot
        node_of_slot[slot] = n
        count[b] += 1
        load[b] += deg[n]
    return slot_of_node, node_of_slot


def prep_host(x, edge_index, n_cores, nb, chunk_group=4):
    """Build per-core tables. Returns dict of host data."""
    n_nodes = x.shape[0]
    n_bins = n_cores * nb
    src, dst = np.asarray(edge_index[0]), np.asarray(edge_index[1])
    slot_of_node, node_of_slot = assign_blocks(dst, n_nodes, n_bins)

    sslot = slot_of_node[src]          # source slot per edge
    dslot = slot_of_node[dst]
    dbin = dslot // 128
    drel = dslot % 128

    # group edges by destination bin
    ord_ = np.argsort(dbin, kind="stable")
    sslot, drel, dbin = sslot[ord_], drel[ord_], dbin[ord_]
    counts = np.bincount(dbin, minlength=n_bins)
    k_chunks = int(np.ceil(counts.max() / 128))
    k_chunks = int(np.ceil(k_chunks / chunk_group) * chunk_group)
    eb = k_chunks * 128                 # padded edges per bin
    e_core = nb * eb                    # edges per core

    src_pad = np.zeros((n_bins, eb), dtype=np.int64)
    drel_pad = np.full((n_bins, eb), -1.0, dtype=np.float32)
    ofs = np.concatenate([[0], np.cumsum(counts)])
    for b in range(n_bins):
        c = counts[b]
        src_pad[b, :c] = sslot[ofs[b]:ofs[b] + c]
        drel_pad[b, :c] = drel[ofs[b]:ofs[b] + c]

    per_core = []
    for c in range(n_cores):
        s = src_pad[c * nb:(c + 1) * nb].reshape(-1)          # [e_core]
        d = drel_pad[c * nb:(c + 1) * nb].reshape(-1)         # [e_core]
        # gather idx: wrapped in 16 partitions, replicated x8
        idx16 = s.astype(np.int16).reshape(-1, 16).T          # [16, e/16]
        idx16 = np.tile(idx16, (8, 1)).copy()                 # [128, e/16]
        # dstrel per chunk, partition-major: [128, n_chunks]
        dcol = d.reshape(-1, 128).T.astype(np.float32).copy() # [128, nchunks]
        per_core.append(dict(idx16=idx16, dcol=dcol))

    return dict(
        slot_of_node=slot_of_node, node_of_slot=node_of_slot,
        k_chunks=k_chunks, e_core=e_core, per_core=per_core,
        n_bins=n_bins,
    )


def pack_weights(inp, meta, n_cores, nb):
    """Pack weights/constants shared by all cores (host-side, numpy).
    Returns dict name -> np.ndarray to be fed as kernel inputs."""
    bf = ml_dtypes.bfloat16
    node_of_slot = meta["node_of_slot"]
    slots = node_of_slot.shape[0]
    x = np.asarray(inp["x"])
    n, in_ch = x.shape

    xs = np.zeros((slots, in_ch), dtype=np.float32)
    valid = node_of_slot >= 0
    xs[valid] = x[node_of_slot[valid]]

    out = {}
    # per-core transposed x slice  [128, nb*128]
    per_core_x = []
    sl_per_core = slots // n_cores
    for c in range(n_cores):
        per_core_x.append(
            np.ascontiguousarray(xs[c * sl_per_core:(c + 1) * sl_per_core].T)
            .astype(bf))
    out["__percore__xT"] = per_core_x

    def b(a):
        return np.asarray(a, dtype=bf)

    for li, (wl, bl, wr, br, att, bias, heads, ch) in enumerate([
        (inp["Wl1"], inp["bl1"], inp["Wr1"], inp["br1"], inp["att1"], inp["bias1"], 8, 32),
        (inp["Wl2"], inp["bl2"], inp["Wr2"], inp["br2"], inp["att2"], inp["bias2"], 8, 32),
        (inp["Wl3"], inp["bl3"], inp["Wr3"], inp["br3"], inp["att3"], inp["bias3"], 1, 64),
    ], start=1):
        wl = np.asarray(wl, np.float32); wr = np.asarray(wr, np.float32)
        d2 = heads * ch
        if li == 3:
            # pad out channels 64 -> 128 for the gl3 gather table
            wl = np.concatenate([wl, np.zeros((wl.shape[0], 128 - d2), np.float32)], 1)

        def kblk(w):
            # [in, out] -> [128, kb, out]
            inch = w.shape[0]
            kb = (inch + 127) // 128
            wp = np.zeros((kb * 128, w.shape[1]), np.float32)
            wp[:inch] = w
            return np.ascontiguousarray(wp.reshape(kb, 128, -1).transpose(1, 0, 2))

        out[f"Wl{li}"] = b(kblk(wl))
        out[f"Wr{li}"] = b(kblk(wr))
        def brd(v, pad_to=None):
            v = np.asarray(v, np.float32).reshape(1, -1)
            if pad_to is not None and v.shape[1] < pad_to:
                v = np.concatenate(
                    [v, np.zeros((1, pad_to - v.shape[1]), np.float32)], 1)
            return np.ascontiguousarray(np.tile(v, (128, 1)))

        out[f"bl{li}"] = brd(bl, 128 if li == 3 else None)
        out[f"br{li}"] = brd(br)
        out[f"obias{li}"] = brd(bias)
        attrow = np.asarray(att, np.float32).reshape(1, d2)
        out[f"att{li}"] = b(np.tile(attrow, (128, 1)))
    out["ident"] = b(np.eye(128, dtype=np.float32))
    out["iota_row"] = np.tile(
        np.arange(128, dtype=np.int16)[None, :], (128, 1)).copy()
    return out


CONST_ORDER = [
    "ident", "iota_row", "xT", "idx16", "dcol",
    "Wl1", "Wr1", "bl1", "br1", "obias1", "att1",
    "Wl2", "Wr2", "bl2", "br2", "obias2", "att2",
    "Wl3", "Wr3", "bl3", "br3", "obias3", "att3",
]


def make_core_inputs(packed, meta, core):
    """Assemble per-core kernel inputs: one const blob.
    Returns (inputs_dict, blob_offsets)."""
    pc = meta["per_core"][core]
    consts = {}
    for name in CONST_ORDER:
        if name == "xT":
            consts[name] = packed["__percore__xT"][core]
        elif name == "idx16":
            consts[name] = pc["idx16"]
        elif name == "dcol":
            consts[name] = pc["dcol"]
        else:
            consts[name] = packed[name]
    blob, offsets = build_blob(consts)
    return {"blob": blob}, offsets


def build_blob(consts):
    """Pack {name: [128, ...] array} into one [128, B] uint8 blob + offset map.
    Each array must have partition dim 128; free dims flattened."""
    offsets = {}
    parts = []
    off = 0
    for name, arr in consts.items():
        assert arr.shape[0] == 128, (name, arr.shape)
        flat = np.ascontiguousarray(arr).reshape(128, -1)
        by = flat.view(np.uint8).reshape(128, -1)
        pad = (-by.shape[1]) % 4
        if pad:
            by = np.concatenate(
                [by, np.zeros((128, pad), np.uint8)], axis=1)
        offsets[name] = (off, arr.dtype, arr.shape[1:])
        parts.append(by)
        off += by.shape[1]
    return np.concatenate(parts, axis=1), offsets


# ---------------------------------------------------------------- kernel

class Cfg:
    def __init__(self, n_cores, nb, k_chunks, grp=4):
        self.n_cores = n_cores
        self.nb = nb                  # blocks per core
        self.k_chunks = k_chunks      # 128-edge chunks per block
        self.grp = grp                # chunks per group
        self.slots = n_cores * nb * 128
        self.own = nb * 128
        self.e_core = nb * k_chunks * 128


def build_kernel(tc, outs, ins, cfg: Cfg):
    nc = tc.nc
    NB, K, G = cfg.nb, cfg.k_chunks, cfg.grp
    NGRP = K // G
    OWN = cfg.own
    SLOTS = cfg.slots

    out_dram = outs["out"]
    IN = 128

    # layer configs: (d2, heads, ch, table_cols, in_ch)
    layers = [
        dict(li=1, heads=8, ch=32, d2=256, tab=256, inch=IN, kb=1),
        dict(li=2, heads=8, ch=32, d2=256, tab=256, inch=256, kb=2),
        dict(li=3, heads=1, ch=64, d2=64, tab=128, inch=256, kb=2),
    ]

    from contextlib import ExitStack
    ctx = ExitStack()
    import os as _os
    _nl = int(_os.environ.get("GAT_LAYERS", "3"))
    _ph = _os.environ.get("GAT_PHASE", "all")
    cc = ctx.enter_context(tc.tile_pool(name="const", bufs=1))
    dram = ctx.enter_context(tc.tile_pool(name="dram", bufs=1, space="DRAM"))
    work = ctx.enter_context(tc.tile_pool(name="work", bufs=2))
    psum = ctx.enter_context(tc.tile_pool(name="psum", bufs=2, space="PSUM"))
    psum_out = ctx.enter_context(
        tc.tile_pool(name="psum_out", bufs=1, space="PSUM"))
    psum_d = ctx.enter_context(tc.tile_pool(name="psum_d", bufs=1, space="PSUM"))
    gath_pool = ctx.enter_context(tc.tile_pool(name="gath", bufs=2))

    # ---------- load all constants with ONE DMA from the packed blob
    np2dt = {
        np.dtype(np.float32): F32,
        np.dtype(np.int16): I16,
        np.dtype("bfloat16"): BF16,
    }
    blob_ap = ins["blob"]
    blob = cc.tile([128, blob_ap.shape[1]], mybir.dt.uint8, tag="blob")
    nc.sync.dma_start(blob[:], blob_ap)

    def cview(name):
        off, dt, shape = cfg.blob_offsets[name]
        dtm = np2dt[np.dtype(dt)]
        n = int(np.prod(shape)) if shape else 1
        v = blob[:, off:off + n * np.dtype(dt).itemsize].bitcast(dtm)
        if len(shape) == 2:
            v = v.rearrange("p (a b) -> p a b", b=shape[1])
        return v

    ident = cview("ident")
    iota_row = cview("iota_row")           # [128,128] i16: value = col
    idx16 = cview("idx16")
    dcol = cview("dcol")
    xT = cview("xT")
    wt = {}
    for l in layers:
        li = l["li"]
        for nm in (f"Wl{li}", f"Wr{li}", f"bl{li}", f"br{li}",
                   f"obias{li}", f"att{li}"):
            wt[nm] = cview(nm)

    # persistent h state (own nodes)
    h_sb = cc.tile([128, NB, 256], BF16, tag="h_sb")
    hT = cc.tile([128, 2, OWN], BF16, tag="hT")
    gr_sb = cc.tile([128, NB, 256], BF16, tag="gr_sb")

    # DRAM: gl shard + allgather output per layer
    gl_shard = {
        l["li"]: dram.tile([OWN, l["tab"]], BF16, name=f"gl_shard{l['li']}")
        for l in layers
    }
    gl_full = {
        l["li"]: dram.tile([SLOTS, l["tab"]], BF16, addr_space="Shared",
                           name=f"gl_full{l['li']}")
        for l in layers
    }

    replica_groups = [list(range(cfg.n_cores))]

    for l in layers[:_nl]:
        li, heads, ch, d2, tab, inch, kb = (
            l["li"], l["heads"], l["ch"], l["d2"], l["tab"], l["inch"], l["kb"])

        # ---------------- dense phase ----------------
        if li > 1:
            # hT <- transpose(h_sb)
            for b in range(NB):
                for k in range(2):
                    pt = psum_d.tile([128, 128], BF16, tag="pt")
                    nc.tensor.transpose(
                        pt[:], h_sb[:, b, k * 128:(k + 1) * 128], ident[:])
                    nc.vector.tensor_copy(
                        hT[:, k, b * 128:(b + 1) * 128], pt[:])

        def lhsT_blk(kbi, b):
            if li == 1:
                return xT[:, b * 128:(b + 1) * 128]
            return hT[:, kbi, b * 128:(b + 1) * 128]

        for b in range(NB):
            for (wn, bn, store_gr) in ((f"Wl{li}", f"bl{li}", False),
                                       (f"Wr{li}", f"br{li}", True)):
                cols = d2 if store_gr else tab
                pg = psum_d.tile([128, 256], F32, tag="pg")
                for kbi in range(kb):
                    nc.tensor.matmul(
                        pg[:, 0:cols], lhsT_blk(kbi, b),
                        wt[wn][:, kbi, 0:cols],
                        start=(kbi == 0), stop=(kbi == kb - 1))
                if store_gr:
                    nc.vector.tensor_tensor(
                        gr_sb[:, b, 0:cols], pg[:, 0:cols],
                        wt[bn][:, 0:cols], ALU.add)
                else:
                    t = work.tile([128, tab], BF16, tag="gl_blk")
                    nc.vector.tensor_tensor(
                        t[:, 0:cols], pg[:, 0:cols],
                        wt[bn][:, 0:cols], ALU.add)
                    nc.sync.dma_start(
                        gl_shard[li][b * 128:(b + 1) * 128, :], t[:])
        # allgather gl table
        nc.gpsimd.collective_compute(
            "AllGather", ALU.bypass,
            ins=[gl_shard[li].opt()], outs=[gl_full[li].opt()],
            replica_groups=replica_groups)

        # ---------------- edge phase ----------------
        att = wt[f"att{li}"]
        obias = wt[f"obias{li}"]
        ech = K * 128 // 16            # idx16 cols per block
        for b in (range(NB) if _ph == "all" else []):
            gt = gath_pool.tile([128, K, tab], BF16, tag="gath")
            # split into <=1024-index sub-gathers: larger single calls
            # (4096 idxs) abort/hang the SWDGE path on this runtime
            GSUB = 8                     # chunks (of 128 edges) per gather
            for gs in range(0, K, GSUB):
                kk = min(GSUB, K - gs)
                nc.gpsimd.dma_gather(
                    gt[:, gs:gs + kk, :], gl_full[li],
                    idx16[:, b * ech + gs * 8:b * ech + (gs + kk) * 8],
                    num_idxs=kk * 128, num_idxs_reg=kk * 128,
                    elem_size=tab, queue_num=0)
            po = psum_out.tile([128, 512], F32, tag="po")
            for g in range(NGRP):
                A4 = work.tile([128, G, 128], BF16, tag="A4")
                AT4 = work.tile([128, G, 128], BF16, tag="AT4")
                ptr = psum_d.tile([128, G, 128], BF16, tag="ptr")
                for j in range(G):
                    ci = b * K + g * G + j      # global chunk index (core)
                    nc.vector.tensor_scalar(
                        A4[:, j, :], iota_row[:],
                        dcol[:, ci:ci + 1], None, op0=ALU.is_equal)
                    nc.tensor.transpose(ptr[:, j, :], A4[:, j, :], ident[:])
                nc.vector.tensor_copy(AT4[:], ptr[:])
                tp = psum.tile([128, G, 256], F32, tag="tp")
                for j in range(G):
                    nc.tensor.matmul(
                        tp[:, j, 0:d2], AT4[:, j, :], gr_sb[:, b, 0:d2],
                        start=True, stop=False)
                    nc.tensor.matmul(
                        tp[:, j, 0:d2], ident[:],
                        gt[:, g * G + j, 0:d2], start=False, stop=True)
                tmpc = work.tile([128, G, 256], BF16, tag="tmpc")
                nc.scalar.activation(tmpc[:, :, 0:d2], tp[:, :, 0:d2], AF.Copy)
                tmp = work.tile([128, G, 256], BF16, tag="tmp")
                nc.vector.scalar_tensor_tensor(
                    tmp[:, :, 0:d2], tmpc[:, :, 0:d2], NEG_SLOPE,
                    tmpc[:, :, 0:d2], op0=ALU.mult, op1=ALU.max)
                t2 = work.tile([128, G, 256], BF16, tag="t2")
                nc.vector.tensor_tensor(
                    t2[:, :, 0:d2], tmp[:, :, 0:d2],
                    att[:, 0:d2].unsqueeze(1).broadcast_to((128, G, d2)),
                    ALU.mult)
                score = work.tile([128, G, 8], F32, tag="score")
                nc.vector.tensor_reduce(
                    score[:, :, 0:heads],
                    t2[:, :, 0:d2].rearrange("p g (h c) -> p g h c", c=ch),
                    axis=AX.X, op=ALU.add)
                ex = work.tile([128, G, 8], BF16, tag="ex")
                nc.scalar.activation(
                    ex[:, :, 0:heads], score[:, :, 0:heads], AF.Exp)
                # rhs = [ gl_src * ex (per-head broadcast) | ex ]
                rhs = work.tile([128, G, 272], BF16, tag="rhs")
                nc.vector.tensor_tensor(
                    rhs[:, :, 0:d2].rearrange("p g (h c) -> p g h c", c=ch),
                    gt[:, g * G:(g + 1) * G, 0:d2].rearrange(
                        "p g (h c) -> p g h c", c=ch),
                    ex[:, :, 0:heads].unsqueeze(3).broadcast_to(
                        (128, G, heads, ch)),
                    ALU.mult)
                nc.vector.tensor_copy(
                    rhs[:, :, d2:d2 + heads], ex[:, :, 0:heads])
                for j in range(G):
                    nc.tensor.matmul(
                        po[:, 0:d2 + heads], A4[:, j, :],
                        rhs[:, j, 0:d2 + heads],
                        start=(g == 0 and j == 0),
                        stop=(g == NGRP - 1 and j == G - 1))
            # -------- block epilogue: normalize + bias (+elu / sigmoid)
            den = work.tile([128, 8], F32, tag="den")
            nc.vector.tensor_scalar(
                den[:, 0:heads], po[:, d2:d2 + heads], 1e-16, None,
                op0=ALU.add)
            rec = work.tile([128, 8], F32, tag="rec")
            nc.vector.reciprocal(rec[:, 0:heads], den[:, 0:heads])
            hx = work.tile([128, 256], F32, tag="hx")
            nc.vector.tensor_tensor(
                hx[:, 0:d2].rearrange("p (h c) -> p h c", c=ch),
                po[:, 0:d2].rearrange("p (h c) -> p h c", c=ch),
                rec[:, 0:heads].unsqueeze(2).broadcast_to((128, heads, ch)),
                ALU.mult)
            nc.vector.tensor_tensor(
                hx[:, 0:d2], hx[:, 0:d2], obias[:, 0:d2], ALU.add)
            if li < 3:
                m0 = work.tile([128, 256], F32, tag="m0")
                nc.vector.tensor_scalar(
                    m0[:, 0:d2], hx[:, 0:d2], 0.0, None, op0=ALU.min)
                e0 = work.tile([128, 256], F32, tag="e0")
                nc.scalar.activation(e0[:, 0:d2], m0[:, 0:d2], AF.Exp)
                nc.vector.scalar_tensor_tensor(
                    h_sb[:, b, 0:d2], e0[:, 0:d2], -1.0, hx[:, 0:d2],
                    op0=ALU.add, op1=ALU.max)
            else:
                so = work.tile([128, 64], F16, tag="so")
                nc.scalar.activation(so[:, 0:d2], hx[:, 0:d2], AF.Sigmoid)
                nc.sync.dma_start(
                    out_dram[b * 128:(b + 1) * 128, :], so[:, 0:d2])
    ctx.close()


# ================================================================ entry point

N_NODES, N_EDGES = 10000, 320000
OUT_CH = 64
N_CORES_K = 8
NB_K = 10


def _build_program(cfg, shapes_dtypes):
    import concourse.bacc as bacc
    nc = bacc.Bacc("TRN2", target_bir_lowering=False, debug=False,
                   enable_asserts=False, num_devices=cfg.n_cores)
    ins = {}
    for name, (shape, dt) in shapes_dtypes.items():
        ins[name] = nc.dram_tensor(name, list(shape), dt, kind="ExternalInput").ap()
    out = nc.dram_tensor("out", [cfg.own, OUT_CH], F16, kind="ExternalOutput").ap()
    with tile.TileContext(nc) as tc:
        build_kernel(tc, {"out": out}, ins, cfg)
    nc.compile()
    return nc


def _make_runner(nc, n_cores, in_maps):
    """Build a cached single-dispatch runner: jitted shard_map around the
    bass_exec custom call, with all inputs device-resident. Returns a
    zero-argument callable -> list of per-core output dicts (host numpy)."""
    import jax
    from jax.sharding import Mesh, PartitionSpec, NamedSharding
    import warnings
    with warnings.catch_warnings():
        warnings.simplefilter("ignore")
        from jax.experimental.shard_map import shard_map
    from concourse.bass2jax import (
        _bass_exec_p, install_neuronx_cc_hook, partition_id_tensor)

    install_neuronx_cc_hook()

    partition_name = (
        nc.partition_id_tensor.name if nc.partition_id_tensor else None)
    in_names, out_names, out_avals, zero_outs = [], [], [], []
    in_shapes = {}
    for alloc in nc.m.functions[0].allocations:
        if not isinstance(alloc, mybir.MemoryLocationSet):
            continue
        name = alloc.memorylocations[0].name
        if alloc.kind == "ExternalInput":
            if name != partition_name:
                in_names.append(name)
                in_shapes[name] = (
                    tuple(alloc.tensor_shape), mybir.dt.np(alloc.dtype))
        elif alloc.kind == "ExternalOutput":
            shape = tuple(alloc.tensor_shape)
            dtype = mybir.dt.np(alloc.dtype)
            out_names.append(name)
            out_avals.append(jax.core.ShapedArray(shape, dtype))
            zero_outs.append(np.zeros(shape, dtype))
    n_params = len(in_names)
    in_names_all = list(in_names) + list(out_names)
    if partition_name is not None:
        in_names_all.append(partition_name)

    def _body(*args):
        operands = list(args)
        if partition_name is not None:
            operands.append(partition_id_tensor())
        outs = _bass_exec_p.bind(
            *operands, out_avals=tuple(out_avals),
            in_names=tuple(in_names_all), out_names=tuple(out_names),
            lowering_input_output_aliases=(), sim_require_finite=True,
            sim_require_nnan=True, nc=nc)
        return tuple(outs)

    devices = jax.devices()[:n_cores]
    mesh = Mesh(np.asarray(devices), ("core",))
    spec = PartitionSpec("core")
    sharded = jax.jit(
        shard_map(_body, mesh=mesh,
                  in_specs=(spec,) * (n_params + len(out_names)),
                  out_specs=(spec,) * len(out_names), check_rep=False),
        keep_unused=True)
    shard = NamedSharding(mesh, spec)

    # device-resident inputs (missing names e.g. dbg tensors -> zeros)
    dev_in = []
    for name in in_names:
        if name in in_maps[0]:
            parts = [np.asarray(in_maps[c][name]) for c in range(n_cores)]
        else:
            sh, dt = in_shapes[name]
            parts = [np.zeros(sh, dt)] * n_cores
        dev_in.append(jax.device_put(np.concatenate(parts, axis=0), shard))
    dev_zero = [
        jax.device_put(
            np.zeros((n_cores * z.shape[0],) + z.shape[1:], z.dtype), shard)
        for z in zero_outs]
    jax.block_until_ready(dev_in + dev_zero)

    def run():
        outs = sharded(*dev_in, *dev_zero)
        arr = np.asarray(outs[0])        # [n_cores*own, OUT_CH]
        return arr

    return run


_STATE = {}
_PREP_CACHE = {}
_PROG_CACHE = {}


def _content_key(arrs):
    h = hashlib.blake2b(digest_size=16)
    for k in sorted(arrs):
        a = arrs[k]
        h.update(k.encode())
        h.update(str(a.shape).encode())
        h.update(str(a.dtype).encode())
        h.update(np.ascontiguousarray(a).tobytes())
    return h.digest()


def kernel(**inputs):
    """Full-input entry: shard across 8 NeuronCores, run, gather."""
    arrs = {k: np.asarray(v) for k, v in inputs.items()}
    key = _content_key(arrs)
    st = _STATE.get("st")
    if st is None or st["key"] != key:
        x = arrs["x"].astype(np.float32, copy=False)
        edge_index = arrs["edge_index"]
        ekey = hash(edge_index.tobytes())
        if ekey in _PREP_CACHE:
            meta = _PREP_CACHE[ekey]
        else:
            meta = prep_host(x, edge_index, N_CORES_K, NB_K)
            _PREP_CACHE.clear()
            _PREP_CACHE[ekey] = meta
        cfg = Cfg(N_CORES_K, NB_K, meta["k_chunks"])
        packed = pack_weights(arrs, meta, N_CORES_K, NB_K)

        in_maps = []
        for c in range(N_CORES_K):
            d, offsets = make_core_inputs(packed, meta, c)
            cfg.blob_offsets = offsets
            in_maps.append(d)

        pkey = (cfg.k_chunks, in_maps[0]["blob"].shape[1])
        if pkey not in _PROG_CACHE:
            shapes_dtypes = {
                name: (arr.shape, mybir.dt.from_np(arr.dtype))
                for name, arr in in_maps[0].items()
            }
            _PROG_CACHE.clear()
            _PROG_CACHE[pkey] = _build_program(cfg, shapes_dtypes)
        nc = _PROG_CACHE[pkey]

        run = _make_runner(nc, N_CORES_K, in_maps)

        nos = meta["node_of_slot"]
        valid = nos >= 0
        scatter_rows = nos[valid]
        st = dict(key=key, run=run, valid=valid, scatter_rows=scatter_rows)
        _STATE["st"] = st

    arr = st["run"]()
    full = np.zeros((N_NODES, OUT_CH), np.float32)
    full[st["scatter_rows"]] = arr[st["valid"]].astype(np.float32)
    return full
